# revision 9
# baseline (speedup 1.0000x reference)
"""Trainium2 Bass kernel for: out_t = silu(cumsum_t(x)) diff along T.

Reference (T, B, L, D) = (4, 2, 2048, 4096) f32:
    Y = silu(cumsum(x, axis=0)); out = concat([Y[:1], Y[1:] - Y[:-1]])

Strategy: shard L across the 8 NeuronCores (embarrassingly parallel; the
scan is over T=4 only).  Per core a raw-Bass 3-stage pipeline streams
chunks of 128x(4x1024) f16 through SBUF:

  SP  : strided 1 MiB HWDGE loads (all 4 t-slices of a chunk at once);
        the first chunk is split into 4 smaller DMAs so all 16 SDMA
        engines ramp up sooner
  DVE : running sums (3 adds) + output diffs (3 subs), all-f16 operands
        so every op runs in the 2x_1p high-rate mode; the adds of chunk
        i+1 are interleaved with the diffs of chunk i so same-engine
        RAW drain-waits never stall the engine
  ACT : silu0 (straight into the out tile) + ONE merged silu over the
        [P, 3F] running-sum tile (halves ACT instruction overhead)
        + 1 MiB HWDGE stores on its own ring; the last chunk loads/
        stores per t-slice to shorten the tail

Explicit semaphores; every dma_start carries zero attached waits (the
DMA ISA encoding only fits one) — cross-engine deps are standalone
sequencer wait_ge instructions.

Both input and output cross HBM as f16 (the host downcasts x and widens
the result back to f32): ~7e-4 l2 rel err, well inside the 2e-2 gate,
cutting HBM traffic from 64 MiB to 32 MiB per core.  The 16 SDMA
engines sustain ~425 GB/s/core, so the DMA floor is ~79 us.
"""

import sys

if "/opt/trn_rl_repo" not in sys.path:
    sys.path.insert(0, "/opt/trn_rl_repo")

import numpy as np

T, B, L, D = 4, 2, 2048, 4096
NCORES = 8
LS = L // NCORES            # 256 rows of L per core
NPOS = B * LS * D           # 2_097_152 elements per t-slice per core
P = 128                     # SBUF partitions
F = 1024                    # free-dim elements per tile slice
NCHUNK = NPOS // (P * F)    # 16 chunk iterations per core
NBUF = 8                    # xb / ob slot count
PP = 2                      # acc / y ping-pong depth

_NC_CACHE = {}
LAST_RESULT = None
TRACE = False
TRACE_CORES = None
TMPDIR = None


def _build_nc(use_silu: bool = True):
    import concourse.bass as bass
    from concourse import mybir

    f16 = mybir.dt.float16
    act_fn = (
        mybir.ActivationFunctionType.Silu
        if use_silu
        else mybir.ActivationFunctionType.Sigmoid
    )

    nc = bass.Bass("TRN2", debug=False)
    # Chunk-major DRAM layout [NCHUNK, P, T, F] (host repacks): each
    # partition's chunk data is one contiguous 8 KiB run, so every DMA
    # is a straight copy with maximal descriptors — no strided
    # t-permute APs.
    x_d = nc.declare_dram_parameter("x", [NCHUNK, P, T, F], f16, isOutput=False)
    o_d = nc.declare_dram_parameter("out", [NCHUNK, P, T, F], f16, isOutput=True)

    xb = [nc.alloc_sbuf_tensor(f"xb{s}", [P, T, F], f16).ap() for s in range(NBUF)]
    ob = [nc.alloc_sbuf_tensor(f"ob{s}", [P, T, F], f16).ap() for s in range(NBUF)]
    # Running sums / silu results for t=1..3 live in ONE flat [P, 3F]
    # tile per ping-pong slot so a single ACT instruction computes all
    # three silus.
    acc = [nc.alloc_sbuf_tensor(f"acc{p}", [P, (T - 1) * F], f16).ap()
           for p in range(PP)]
    y = [nc.alloc_sbuf_tensor(f"y{p}", [P, (T - 1) * F], f16).ap()
         for p in range(PP)]

    def col(ap, t):  # t-th F-wide column of a flat [P, 3F] tile
        return ap[:, t * F:(t + 1) * F]

    import contextlib

    with contextlib.ExitStack() as es:
        block = es.enter_context(nc.Block())
        # One load/store sem lane per buffer slot: a lane's next DMA never
        # overlaps its previous one (slot-reuse waits guarantee it), so the
        # ">= 16*n" threshold semantics stay sound.
        s_load = [es.enter_context(nc.semaphore(f"s_load{k}")) for k in range(NBUF)]
        s_store = [es.enter_context(nc.semaphore(f"s_store{k}")) for k in range(NBUF)]
        s_acc = es.enter_context(nc.semaphore("s_acc"))
        s_act = es.enter_context(nc.semaphore("s_act"))
        s_out = es.enter_context(nc.semaphore("s_out"))
        # Dedicated per-slice sems for the split first-chunk load and the
        # split last-chunk load/store (one DMA per sem keeps every
        # threshold sound).
        s_l0 = [es.enter_context(nc.semaphore(f"s_l0_{t}")) for t in range(T)]
        s_ll = [es.enter_context(nc.semaphore(f"s_ll{t}")) for t in range(T)]
        s_ls = [es.enter_context(nc.semaphore(f"s_ls{t}")) for t in range(T)]
        LAST = NCHUNK - 1

        def ld_lane(i):
            assert i != LAST and i != 0
            return s_load[i % NBUF], 16 * (i // NBUF + (1 if i % NBUF else 0))

        def st_lane(i):
            assert i != LAST
            return s_store[i % NBUF], 16 * (i // NBUF + 1)

        @block.sync
        def _(sp: bass.BassEngine):
            for i in range(NCHUNK):
                if i >= NBUF:
                    j = i - NBUF
                    # xb slot free: DVE adds + ACT silu0 of chunk j done.
                    # (These also transitively cover load j's completion, so
                    # this lane's previous inc is observed before re-use.)
                    sp.wait_ge(s_acc, 3 * (j + 1))
                    sp.wait_ge(s_act, 2 * j + 1)
                if i == 0:
                    # split: smaller first DMAs reach all 16 SDMA engines
                    # (esp. the late-starting ones) sooner
                    for t in range(T):
                        sp.dma_start(
                            out=xb[0][:, t], in_=x_d[0][:, t]
                        ).then_inc(s_l0[t], 16)
                elif i == LAST:
                    # split: per-slice sems let compute start per slice
                    for t in range(T):
                        sp.dma_start(
                            out=xb[i % NBUF][:, t], in_=x_d[i][:, t]
                        ).then_inc(s_ll[t], 16)
                else:
                    sem, _v = ld_lane(i)
                    sp.dma_start(
                        out=xb[i % NBUF][:], in_=x_d[i]
                    ).then_inc(sem, 16)

        @block.vector
        def _(ve: bass.BassEngine):
            def wait_slice(i, t):
                # load of chunk i's t-th slice complete (full-chunk lane
                # sem for middle chunks; per-slice sems at the ends)
                if i == 0:
                    ve.wait_ge(s_l0[t], 16)
                elif i == LAST:
                    ve.wait_ge(s_ll[t], 16)
                elif t == 0:
                    ve.wait_ge(*ld_lane(i))

            def emit_add(i, k):
                # k-th of the 3 running-sum adds for chunk i
                xs, a = i % NBUF, acc[i % PP]
                if k == 0:
                    wait_slice(i, 0)
                    wait_slice(i, 1)
                    if i >= PP:
                        # acc slot free: merged silu of chunk i-PP done
                        ve.wait_ge(s_act, 2 * (i - PP) + 2)
                    ve.tensor_add(
                        col(a, 0), xb[xs][:, 0], xb[xs][:, 1]
                    ).then_inc(s_acc)
                else:
                    # same-engine RAW on the acc chain needs a drain-backed
                    # sem wait (interleaved subs below hide the latency)
                    ve.wait_ge(s_acc, 3 * i + k)
                    wait_slice(i, k + 1)
                    ve.tensor_add(
                        col(a, k), col(a, k - 1), xb[xs][:, k + 1]
                    ).then_inc(s_acc)

            def emit_sub(i, k):
                # k-th of the 3 output diffs for chunk i; sub1 reads the
                # f16 y0 slice ACT wrote into ob directly.  ACT's silu0
                # already waited for this ob slot to drain, so s_act
                # covers slot-reuse transitively.
                os_, yy = i % NBUF, y[i % PP]
                if k == 0:
                    ve.wait_ge(s_act, 2 * i + 2)  # merged silu of i done
                    ve.tensor_sub(
                        ob[os_][:, 1], col(yy, 0), ob[os_][:, 0]
                    ).then_inc(s_out)
                else:
                    ve.tensor_sub(
                        ob[os_][:, k + 1], col(yy, k), col(yy, k - 1)
                    ).then_inc(s_out)

            # Software-pipelined + interleaved: the adds of chunk i+1 are
            # interwoven with the diffs of chunk i, so each add's RAW
            # drain-wait lands after ~0.5us of unrelated sub work.
            for k in range(3):
                emit_add(0, k)
            for i in range(NCHUNK):
                if i + 1 < NCHUNK:
                    emit_add(i + 1, 0)
                    emit_sub(i, 0)
                    emit_add(i + 1, 1)
                    emit_sub(i, 1)
                    emit_add(i + 1, 2)
                    emit_sub(i, 2)
                else:
                    for k in range(3):
                        emit_sub(i, k)

        @block.scalar
        def _(se: bass.BassEngine):
            # ACT does the silus AND issues the stores on its own HWDGE ring
            # (qActDynamicHW) — keeps GpSimd DMA-free so the end-of-block
            # dge_drain has nothing to drain.
            for i in range(NCHUNK):
                xs, os_, ps = i % NBUF, i % NBUF, i % PP
                if i == LAST:
                    se.wait_ge(s_ll[0], 16)  # reads xb[:,0]
                elif i == 0:
                    se.wait_ge(s_l0[0], 16)
                else:
                    se.wait_ge(*ld_lane(i))
                if i >= NBUF:
                    se.wait_ge(*st_lane(i - NBUF))  # ob slot free
                se.activation(ob[os_][:, 0], xb[xs][:, 0], act_fn).then_inc(s_act)
                if i == LAST:
                    # per-slice stores: each output slice leaves as soon as
                    # it's ready, shrinking the end-of-kernel critical path
                    se.wait_ge(s_act, 2 * i + 1)  # own silu0 drained
                    se.dma_start(out=o_d[i][:, 0], in_=ob[os_][:, 0]).then_inc(
                        s_ls[0], 16
                    )
                # ONE merged silu for t=1..3 over the flat [P, 3F] acc tile
                se.wait_ge(s_acc, 3 * i + 3)          # all adds of i done
                if i >= PP:
                    se.wait_ge(s_out, 3 * (i - PP + 1))  # y slot free
                se.activation(y[ps][:], acc[ps][:], act_fn).then_inc(s_act)
                if i == LAST:
                    for t in range(1, T):
                        se.wait_ge(s_out, 3 * i + t)
                        se.dma_start(
                            out=o_d[i][:, t], in_=ob[os_][:, t]
                        ).then_inc(s_ls[t], 16)
                else:
                    # store chunk i once DVE's diffs are done
                    se.wait_ge(s_out, 3 * (i + 1))
                    sem, _v = st_lane(i)
                    if i >= NBUF:
                        # observe this lane's previous store before re-inc'ing
                        se.wait_ge(s_store[i % NBUF], 16 * (i // NBUF))
                    se.dma_start(
                        out=o_d[i], in_=ob[i % NBUF][:]
                    ).then_inc(sem, 16)
            for k in range(NBUF):
                n_regular = len([i for i in range(NCHUNK) if i % NBUF == k and i != LAST])
                se.wait_ge(s_store[k], 16 * n_regular)
            for t in range(T):
                se.wait_ge(s_ls[t], 16)

    return nc


def get_nc(use_silu: bool = True):
    key = ("nc", use_silu)
    if key not in _NC_CACHE:
        _NC_CACHE[key] = _build_nc(use_silu)
    return _NC_CACHE[key]


def kernel(x: np.ndarray) -> np.ndarray:
    global LAST_RESULT
    from concourse.bass_utils import run_bass_kernel_spmd

    nc = get_nc()
    x = np.asarray(x, dtype=np.float32).astype(np.float16)
    # repack each core's shard to the chunk-major [NCHUNK, P, T, F] DRAM
    # layout the kernel uses (contiguous per-partition DMA runs)
    in_maps = [
        {"x": np.ascontiguousarray(
            x[:, :, c * LS : (c + 1) * LS, :]
            .reshape(T, NCHUNK, P, F)
            .transpose(1, 2, 0, 3)
        )}
        for c in range(NCORES)
    ]
    try:
        res = run_bass_kernel_spmd(
            nc, in_maps, list(range(NCORES)), trace=TRACE, tmpdir=TMPDIR,
            trace_cores=TRACE_CORES,
        )
    except Exception:
        # rare transient NRT_EXEC_UNIT_UNRECOVERABLE; the device recovers
        # on the next execution
        res = run_bass_kernel_spmd(
            nc, in_maps, list(range(NCORES)), trace=TRACE, tmpdir=TMPDIR,
            trace_cores=TRACE_CORES,
        )
    LAST_RESULT = res
    outs = [
        np.asarray(res.results[c]["out"], dtype=np.float32)
        .transpose(2, 0, 1, 3)
        .reshape(T, B, LS, D)
        for c in range(NCORES)
    ]
    return np.concatenate(outs, axis=2)


# revision 11
# speedup vs baseline: 1.4574x; 1.4574x over previous
"""Trainium2 Bass kernel for: out_t = silu(cumsum_t(x)) diff along T.

Reference (T, B, L, D) = (4, 2, 2048, 4096) f32:
    Y = silu(cumsum(x, axis=0)); out = concat([Y[:1], Y[1:] - Y[:-1]])

Strategy: shard L across the 8 NeuronCores (embarrassingly parallel; the
scan is over T=4 only).  Per core a raw-Bass 3-stage pipeline streams
chunks of 128x(4x1024) f16 through SBUF:

  SP  : strided 1 MiB HWDGE loads (all 4 t-slices of a chunk at once);
        the first chunk is split into 4 smaller DMAs so all 16 SDMA
        engines ramp up sooner
  DVE : running sums (3 adds) + output diffs (3 subs), all-f16 operands
        so every op runs in the 2x_1p high-rate mode; the adds of chunk
        i+1 are interleaved with the diffs of chunk i so same-engine
        RAW drain-waits never stall the engine
  ACT : silu0 (straight into the out tile) + ONE merged silu over the
        [P, 3F] running-sum tile (halves ACT instruction overhead)
        + 1 MiB HWDGE stores on its own ring; the last chunk loads/
        stores per t-slice to shorten the tail

Explicit semaphores; every dma_start carries zero attached waits (the
DMA ISA encoding only fits one) — cross-engine deps are standalone
sequencer wait_ge instructions.

Both input and output cross HBM as f16 (the host downcasts x and widens
the result back to f32): ~7e-4 l2 rel err, well inside the 2e-2 gate,
cutting HBM traffic from 64 MiB to 32 MiB per core.  The 16 SDMA
engines sustain ~425 GB/s/core, so the DMA floor is ~79 us.
"""

import sys

if "/opt/trn_rl_repo" not in sys.path:
    sys.path.insert(0, "/opt/trn_rl_repo")

import numpy as np

T, B, L, D = 4, 2, 2048, 4096
NCORES = 8
LS = L // NCORES            # 256 rows of L per core
NPOS = B * LS * D           # 2_097_152 elements per t-slice per core
P = 128                     # SBUF partitions
F = 1024                    # free-dim elements per tile slice
NCHUNK = NPOS // (P * F)    # 16 chunk iterations per core
NBUF = 8                    # xb / ob slot count
PP = 2                      # acc / y ping-pong depth

_NC_CACHE = {}
LAST_RESULT = None
TRACE = False
TRACE_CORES = None
TMPDIR = None


def _build_nc(use_silu: bool = True):
    import concourse.bass as bass
    from concourse import mybir

    f16 = mybir.dt.float16
    act_fn = (
        mybir.ActivationFunctionType.Silu
        if use_silu
        else mybir.ActivationFunctionType.Sigmoid
    )

    nc = bass.Bass("TRN2", debug=False)
    # Chunk-major DRAM layout [NCHUNK, P, T, F] (host repacks): each
    # partition's chunk data is one contiguous 8 KiB run, so every DMA
    # is a straight copy with maximal descriptors — no strided
    # t-permute APs.
    x_d = nc.declare_dram_parameter("x", [NCHUNK, P, T, F], f16, isOutput=False)
    o_d = nc.declare_dram_parameter("out", [NCHUNK, P, T, F], f16, isOutput=True)

    xb = [nc.alloc_sbuf_tensor(f"xb{s}", [P, T, F], f16).ap() for s in range(NBUF)]
    ob = [nc.alloc_sbuf_tensor(f"ob{s}", [P, T, F], f16).ap() for s in range(NBUF)]
    # Running sums / silu results for t=1..3 live in ONE flat [P, 3F]
    # tile per ping-pong slot so a single ACT instruction computes all
    # three silus.
    acc = [nc.alloc_sbuf_tensor(f"acc{p}", [P, (T - 1) * F], f16).ap()
           for p in range(PP)]
    y = [nc.alloc_sbuf_tensor(f"y{p}", [P, (T - 1) * F], f16).ap()
         for p in range(PP)]

    def col(ap, t):  # t-th F-wide column of a flat [P, 3F] tile
        return ap[:, t * F:(t + 1) * F]

    import contextlib

    with contextlib.ExitStack() as es:
        block = es.enter_context(nc.Block())
        # One load/store sem lane per buffer slot: a lane's next DMA never
        # overlaps its previous one (slot-reuse waits guarantee it), so the
        # ">= 16*n" threshold semantics stay sound.
        s_load = [es.enter_context(nc.semaphore(f"s_load{k}")) for k in range(NBUF)]
        s_store = [es.enter_context(nc.semaphore(f"s_store{k}")) for k in range(NBUF)]
        s_acc = es.enter_context(nc.semaphore("s_acc"))
        s_act = es.enter_context(nc.semaphore("s_act"))
        s_out = es.enter_context(nc.semaphore("s_out"))
        # Dedicated per-slice sems for the split first-chunk load and the
        # split last-chunk load/store (one DMA per sem keeps every
        # threshold sound).
        s_l0 = [es.enter_context(nc.semaphore(f"s_l0_{t}")) for t in range(T)]
        s_ll = [es.enter_context(nc.semaphore(f"s_ll{t}")) for t in range(T)]
        s_ls = [es.enter_context(nc.semaphore(f"s_ls{t}")) for t in range(T)]
        LAST = NCHUNK - 1

        def ld_lane(i):
            assert i != LAST and i != 0
            return s_load[i % NBUF], 16 * (i // NBUF + (1 if i % NBUF else 0))

        def st_lane(i):
            assert i != LAST
            return s_store[i % NBUF], 16 * (i // NBUF + 1)

        @block.sync
        def _(sp: bass.BassEngine):
            for i in range(NCHUNK):
                if i >= NBUF:
                    j = i - NBUF
                    # xb slot free: DVE adds + ACT silu0 of chunk j done.
                    # (These also transitively cover load j's completion, so
                    # this lane's previous inc is observed before re-use.)
                    sp.wait_ge(s_acc, 3 * (j + 1))
                    sp.wait_ge(s_act, 2 * j + 1)
                if i == 0:
                    # split: smaller first DMAs reach all 16 SDMA engines
                    # (esp. the late-starting ones) sooner
                    for t in range(T):
                        sp.dma_start(
                            out=xb[0][:, t], in_=x_d[0][:, t]
                        ).then_inc(s_l0[t], 16)
                elif i == LAST:
                    # split: per-slice sems let compute start per slice
                    for t in range(T):
                        sp.dma_start(
                            out=xb[i % NBUF][:, t], in_=x_d[i][:, t]
                        ).then_inc(s_ll[t], 16)
                else:
                    sem, _v = ld_lane(i)
                    sp.dma_start(
                        out=xb[i % NBUF][:], in_=x_d[i]
                    ).then_inc(sem, 16)

        @block.vector
        def _(ve: bass.BassEngine):
            def wait_slice(i, t):
                # load of chunk i's t-th slice complete (full-chunk lane
                # sem for middle chunks; per-slice sems at the ends)
                if i == 0:
                    ve.wait_ge(s_l0[t], 16)
                elif i == LAST:
                    ve.wait_ge(s_ll[t], 16)
                elif t == 0:
                    ve.wait_ge(*ld_lane(i))

            def emit_add(i, k):
                # k-th of the 3 running-sum adds for chunk i
                xs, a = i % NBUF, acc[i % PP]
                if k == 0:
                    wait_slice(i, 0)
                    wait_slice(i, 1)
                    if i >= PP:
                        # acc slot free: merged silu of chunk i-PP done
                        ve.wait_ge(s_act, 2 * (i - PP) + 2)
                    ve.tensor_add(
                        col(a, 0), xb[xs][:, 0], xb[xs][:, 1]
                    ).then_inc(s_acc)
                else:
                    # same-engine RAW on the acc chain needs a drain-backed
                    # sem wait (interleaved subs below hide the latency)
                    ve.wait_ge(s_acc, 3 * i + k)
                    wait_slice(i, k + 1)
                    ve.tensor_add(
                        col(a, k), col(a, k - 1), xb[xs][:, k + 1]
                    ).then_inc(s_acc)

            def emit_sub(i, k):
                # k-th of the 3 output diffs for chunk i; sub1 reads the
                # f16 y0 slice ACT wrote into ob directly.  ACT's silu0
                # already waited for this ob slot to drain, so s_act
                # covers slot-reuse transitively.
                os_, yy = i % NBUF, y[i % PP]
                if k == 0:
                    ve.wait_ge(s_act, 2 * i + 2)  # merged silu of i done
                    ve.tensor_sub(
                        ob[os_][:, 1], col(yy, 0), ob[os_][:, 0]
                    ).then_inc(s_out)
                else:
                    ve.tensor_sub(
                        ob[os_][:, k + 1], col(yy, k), col(yy, k - 1)
                    ).then_inc(s_out)

            # Software-pipelined order A0, A1, B0, A2, B1, ..., A15, B14,
            # B15: the adds of chunk i+1 run while ACT silus chunk i (their
            # RAW drain-waits hide inside the 3.2us merged silu), so the
            # diffs' s_act wait is already satisfied when reached.
            for k in range(3):
                emit_add(0, k)
            for i in range(NCHUNK):
                if i + 1 < NCHUNK:
                    for k in range(3):
                        emit_add(i + 1, k)
                for k in range(3):
                    emit_sub(i, k)

        @block.scalar
        def _(se: bass.BassEngine):
            # ACT does the silus AND issues the stores on its own HWDGE ring
            # (qActDynamicHW) — keeps GpSimd DMA-free so the end-of-block
            # dge_drain has nothing to drain.  The store of chunk i is
            # issued one iteration LATE (after the silus of chunk i+1): by
            # then DVE's diffs of chunk i are long done, so ACT never
            # stalls waiting on the vector engine — the silu stream runs
            # back-to-back and sets the pipeline pace.
            def emit_store(i):
                # store chunk i once DVE's diffs are done
                se.wait_ge(s_out, 3 * (i + 1))
                sem, _v = st_lane(i)
                if i >= NBUF:
                    # observe this lane's previous store before re-inc'ing
                    se.wait_ge(s_store[i % NBUF], 16 * (i // NBUF))
                se.dma_start(out=o_d[i], in_=ob[i % NBUF][:]).then_inc(sem, 16)

            for i in range(NCHUNK):
                xs, os_, ps = i % NBUF, i % NBUF, i % PP
                if i == LAST:
                    se.wait_ge(s_ll[0], 16)  # reads xb[:,0]
                elif i == 0:
                    se.wait_ge(s_l0[0], 16)
                else:
                    se.wait_ge(*ld_lane(i))
                if i >= NBUF:
                    se.wait_ge(*st_lane(i - NBUF))  # ob slot free
                se.activation(ob[os_][:, 0], xb[xs][:, 0], act_fn).then_inc(s_act)
                if i == LAST:
                    # per-slice stores: each output slice leaves as soon as
                    # it's ready, shrinking the end-of-kernel critical path
                    se.wait_ge(s_act, 2 * i + 1)  # own silu0 drained
                    se.dma_start(out=o_d[i][:, 0], in_=ob[os_][:, 0]).then_inc(
                        s_ls[0], 16
                    )
                # ONE merged silu for t=1..3 over the flat [P, 3F] acc tile
                se.wait_ge(s_acc, 3 * i + 3)          # all adds of i done
                if i >= PP:
                    se.wait_ge(s_out, 3 * (i - PP + 1))  # y slot free
                se.activation(y[ps][:], acc[ps][:], act_fn).then_inc(s_act)
                if i >= 1:
                    emit_store(i - 1)
                if i == LAST:
                    for t in range(1, T):
                        se.wait_ge(s_out, 3 * i + t)
                        se.dma_start(
                            out=o_d[i][:, t], in_=ob[os_][:, t]
                        ).then_inc(s_ls[t], 16)
            for k in range(NBUF):
                n_regular = len([i for i in range(NCHUNK) if i % NBUF == k and i != LAST])
                se.wait_ge(s_store[k], 16 * n_regular)
            for t in range(T):
                se.wait_ge(s_ls[t], 16)

    return nc


def get_nc(use_silu: bool = True):
    key = ("nc", use_silu)
    if key not in _NC_CACHE:
        _NC_CACHE[key] = _build_nc(use_silu)
    return _NC_CACHE[key]


def kernel(x: np.ndarray) -> np.ndarray:
    global LAST_RESULT
    from concourse.bass_utils import run_bass_kernel_spmd

    nc = get_nc()
    x = np.asarray(x, dtype=np.float32).astype(np.float16)
    # repack each core's shard to the chunk-major [NCHUNK, P, T, F] DRAM
    # layout the kernel uses (contiguous per-partition DMA runs)
    in_maps = [
        {"x": np.ascontiguousarray(
            x[:, :, c * LS : (c + 1) * LS, :]
            .reshape(T, NCHUNK, P, F)
            .transpose(1, 2, 0, 3)
        )}
        for c in range(NCORES)
    ]
    try:
        res = run_bass_kernel_spmd(
            nc, in_maps, list(range(NCORES)), trace=TRACE, tmpdir=TMPDIR,
            trace_cores=TRACE_CORES,
        )
    except Exception:
        # rare transient NRT_EXEC_UNIT_UNRECOVERABLE; the device recovers
        # on the next execution
        res = run_bass_kernel_spmd(
            nc, in_maps, list(range(NCORES)), trace=TRACE, tmpdir=TMPDIR,
            trace_cores=TRACE_CORES,
        )
    LAST_RESULT = res
    outs = [
        np.asarray(res.results[c]["out"], dtype=np.float32)
        .transpose(2, 0, 1, 3)
        .reshape(T, B, LS, D)
        for c in range(NCORES)
    ]
    return np.concatenate(outs, axis=2)
